# revision 1
# baseline (speedup 1.0000x reference)
"""Trainium2 Bass kernel for nn_CVNonGaussianQuantumLayer.

12-qubit batched state-vector simulator, batch 128, two circuits
(X-measured and Z-measured). Data-parallel over 8 cores: 16 batch rows
per core; each core simulates its rows for BOTH circuits (32 states).

The metric is end-to-end call latency, and the axon tunnel to the
devices has ~80ms dispatch floor + ~40-100MB/s bandwidth, so the design
goal is minimal per-call transfer:
  - per-call upload: ONE packed f32 tensor dyn [128, 512] per core
    (~256KB): per-gate diagonal columns, packed 32x32 M2 blocks,
    initial-state Kronecker factors (hi [16,128], lo [16,32]).
  - constants (identity, Hadamard, bit-flip perms X_w, CNOT chain,
    sign-reduction matrices, replication matrix) are committed to the
    devices ONCE and passed as device-resident jax arrays (no per-call
    transfer).
  - the 128x128 circuit matrices A = H^T are built ON DEVICE from the
    diagonal columns: each gate G^T = D_a + D'.X_w, so
    T <- da*T + d'*(X_w @ T)  (per-partition tensor_scalar + constant
    permutation matmuls).
  - M2 (32x32) blocks are replicated down partitions with a constant
    replication matmul and applied as 32-partition matmuls per s4 group.
  - measurement sign-reductions run on device (SHX/SL20 matmuls +
    tensor_reduce) -> output is ONE [32, 40] tile per core (~5KB).

Layouts (unchanged from the validated baseline):
  - layout A: partitions = 7 hi bits h, free = (s, lo) with s = c*16+n.
  - layout B (after PE 128-block transposes): partitions = (s mod 4, lo),
    free = (s//4, h).
"""
import sys
import numpy as np

if '/opt/trn_rl_repo' not in sys.path:
    sys.path.insert(0, '/opt/trn_rl_repo')

NQ, NL = 12, 2
NCORES, BPC = 8, 16
NHI, NLO = 7, 5
DHI, DLO = 128, 32

# dyn column layout
C_HI = 0          # [16, 128]   hi factors (partition r, col h)
C_LO = 128        # [16, 32]    lo factors
C_M2 = 160        # 192 cols: M2 pack, 3 bands (base 0/32/64) x 6 slots of 32x32
C_A = 352         # 184 cols: 4 blocks of 46 ((L,c) = (0,0),(0,1),(1,0),(1,1))
C_R1 = 536        # 12 cols: per (L,c): cos, sin, nsin
NDYN = 576

# cpack column layout
CP_SHX = 512      # [128, 8]
CP_SL20 = 520     # [128, 20]
CP_IDG = 544      # [128, 512] IDG: IDG[r, 128*g + p] = d(a(p),r%32)*d(q(p),g)
NCPK = 1056

NWC = 11          # wconst slots
WIDX = dict(ident=0, Hhi=1, CHAINT=2, P56=3, X0=4, X1=5, X2=6, X3=7, X4=8,
            X5=9, X6=10)


# ---------------- host math ----------------
def _rx(th):
    h = 0.5 * th
    return np.array([[np.cos(h), -1j * np.sin(h)], [-1j * np.sin(h), np.cos(h)]])


def _ry(th):
    h = 0.5 * th
    return np.array([[np.cos(h), -np.sin(h)], [np.sin(h), np.cos(h)]])


def _rz(th):
    e = np.exp(-0.5j * th)
    return np.array([[e, 0], [0, np.conj(e)]])


def _phase(phi):
    return np.array([[1, 0], [0, np.exp(1j * phi)]])


def _sigmoid(v):
    return 1.0 / (1.0 + np.exp(-v))


def _fused_u(r3, t1):
    return _phase(_sigmoid(t1) * np.pi) @ _rz(r3[2]) @ _ry(r3[1]) @ _rx(r3[0])


def _kron_at(U, w, n):
    M = np.eye(1, dtype=complex)
    for k in range(n):
        M = np.kron(M, U if k == w else np.eye(2))
    return M


def _kron2_at(U4, w, n):
    M = np.eye(1, dtype=complex)
    k = 0
    while k < n:
        if k == w:
            M = np.kron(M, U4)
            k += 2
        else:
            M = np.kron(M, np.eye(2))
            k += 1
    return M


def _crx4(th):
    U = np.eye(4, dtype=complex)
    U[2:, 2:] = _rx(th)
    return U


_CNOT4 = np.array([[1, 0, 0, 0], [0, 1, 0, 0], [0, 0, 0, 1], [0, 0, 1, 0]],
                  dtype=complex)


def _apply_left_1q(R, U, w):
    # R <- kron_at(U, w, 5) @ R without materializing the kron
    Rv = R.reshape(2 ** w, 2, -1)
    return np.einsum('ij,ajb->aib', U, Rv).reshape(DLO, DLO)


def _apply_left_2q(R, U4, w):
    Rv = R.reshape(2 ** w, 4, -1)
    return np.einsum('ij,ajb->aib', U4, Rv).reshape(DLO, DLO)


_LO_CONST = {}


def _lo_consts():
    if not _LO_CONST:
        chainlo = np.eye(DLO, dtype=complex)
        for w in range(4):
            chainlo = _kron2_at(_CNOT4, w, NLO) @ chainlo
        X7 = _kron_at(np.array([[0, 1], [1, 0]], dtype=complex), 0, NLO)
        _LO_CONST['chain'] = chainlo
        _LO_CONST['chainX7'] = chainlo @ X7
        _LO_CONST['had'] = _hadamards()
    return _LO_CONST


def _lo_mats(rot, cx, t):
    cc = _lo_consts()
    out = []
    for L in range(NL):
        R = np.eye(DLO, dtype=complex)
        for w in range(NHI, NQ):
            R = _apply_left_1q(R, _fused_u(rot[L, w], t[L, w]), w - NHI)
            if w <= NQ - 2:
                R = _apply_left_2q(R, _crx4(_sigmoid(cx[L, w]) * np.pi),
                                   w - NHI)
        out.append(dict(M2_0=cc['chain'] @ R, M2_1=cc['chainX7'] @ R))
    return out


def _hadamards():
    Hd = np.array([[1, 1], [1, -1]], dtype=complex) / np.sqrt(2)
    Hhi = np.eye(1, dtype=complex)
    Hlo = np.eye(1, dtype=complex)
    for _ in range(NHI):
        Hhi = np.kron(Hhi, Hd)
    for _ in range(NLO):
        Hlo = np.kron(Hlo, Hd)
    return Hhi, Hlo


def build_constants():
    Hhi, _ = _hadamards()
    CH = np.eye(DHI, dtype=complex)
    for w in range(5):
        CH = _kron2_at(_CNOT4, w, NHI) @ CH
    CHAINT = np.ascontiguousarray(CH.real.T, dtype=np.float32)
    X = []
    for w in range(NHI):
        X.append(np.ascontiguousarray(
            _kron_at(np.array([[0, 1], [1, 0]], dtype=complex), w, NHI).real,
            dtype=np.float32))
    P56 = np.ascontiguousarray(
        np.kron(np.eye(4), np.kron(np.array([[0., 1.], [1., 0.]]), np.eye(16))),
        dtype=np.float32)
    ident = np.eye(DHI, dtype=np.float32)
    wconst = np.stack([ident, np.ascontiguousarray(Hhi.real, np.float32),
                       CHAINT, P56] + X)
    cpack = np.zeros((128, NCPK), dtype=np.float32)
    m16 = np.zeros((16, 16, 32), np.float32)
    for r in range(16):
        m16[r, r, :] = 1.0
    cpack[:16, 0:512] = m16.reshape(16, 512)
    p = np.arange(128)
    for w in range(NHI):
        cpack[:, CP_SHX + w] = 1.0 - 2.0 * ((p >> (6 - w)) & 1)
    s4, l = p >> 5, p & 31
    for g4 in range(4):
        for wp in range(5):
            cpack[:, CP_SL20 + g4 * 5 + wp] = np.where(
                s4 == g4, 1.0 - 2.0 * ((l >> (4 - wp)) & 1), 0.0)
    for r in range(128):
        for g in range(4):
            cpack[r, CP_IDG + 128 * g + 32 * g + (r % 32)] = 1.0
    return wconst, cpack


def host_prep(x, rotations, cx_strengths, t_gates):
    x = np.asarray(x, np.float64)
    rot = np.asarray(rotations, np.float64)
    cx = np.asarray(cx_strengths, np.float64)
    t = np.asarray(t_gates, np.float64)
    _, Hlo = _lo_consts()['had']

    dyn_shared = np.zeros((128, NDYN), dtype=np.float32)

    lomats = [_lo_mats(rot[c], cx[c], t[c]) for c in range(2)]
    lomats[0][NL - 1]['M2_0'] = Hlo @ lomats[0][NL - 1]['M2_0']
    lomats[0][NL - 1]['M2_1'] = Hlo @ lomats[0][NL - 1]['M2_1']
    bi = 0
    for L in range(NL):
        for c in range(2):
            for b6 in (0, 1):
                for M in (lomats[c][L][f'M2_{b6}'].real,
                          lomats[c][L][f'M2_{b6}'].imag):
                    g, gp = bi // 6, bi % 6
                    dyn_shared[32 * g:32 * g + 32,
                               C_M2 + 32 * gp:C_M2 + 32 * gp + 32] = \
                        np.ascontiguousarray(M.T, np.float32)
                    bi += 1
    p = np.arange(128)
    col = C_A
    for L in range(NL):
        for c in range(2):
            for j in range(13):
                if j % 2 == 0:
                    w = 6 - j // 2
                    U = _fused_u(rot[c, L, w], t[c, L, w])
                    bp = (p >> (6 - w)) & 1
                    da = U[bp, bp]
                    dp = U[1 - bp, bp]
                    dyn_shared[:, col + 0] = da.real
                    dyn_shared[:, col + 1] = da.imag
                    dyn_shared[:, col + 2] = dp.real
                    dyn_shared[:, col + 3] = dp.imag
                    col += 4
                else:
                    w = 5 - j // 2
                    th = _sigmoid(cx[c, L, w]) * np.pi
                    bc = (p >> (6 - w)) & 1
                    dyn_shared[:, col + 0] = np.where(bc, np.cos(0.5 * th), 1.0)
                    dyn_shared[:, col + 1] = -np.sin(0.5 * th) * bc
                    dyn_shared[:, col + 2] = np.sin(0.5 * th) * bc
                    col += 3
    for L in range(NL):
        for c in range(2):
            th67 = _sigmoid(cx[c, L, 6]) * np.pi
            k = C_R1 + 3 * (2 * L + c)
            dyn_shared[:, k + 0] = np.cos(0.5 * th67)
            dyn_shared[:, k + 1] = np.sin(0.5 * th67)
            dyn_shared[:, k + 2] = -np.sin(0.5 * th67)

    ang = np.arctan2(x, 1.0) * np.pi
    h = 0.5 * ang
    cth, sth = np.cos(h), np.sin(h)
    hi = np.ones((x.shape[0], 1))
    for w in range(NHI):
        vec = np.stack([cth[:, w], sth[:, w]], axis=-1)
        hi = (hi[:, :, None] * vec[:, None, :]).reshape(x.shape[0], -1)
    lo = np.ones((x.shape[0], 1))
    for w in range(NHI, NQ):
        vec = np.stack([cth[:, w], sth[:, w]], axis=-1)
        lo = (lo[:, :, None] * vec[:, None, :]).reshape(x.shape[0], -1)

    dyn = np.broadcast_to(dyn_shared, (NCORES, 128, NDYN)).copy()
    hi32 = hi.astype(np.float32).reshape(NCORES, BPC, 128)
    lo32 = lo.astype(np.float32).reshape(NCORES, BPC, 32)
    dyn[:, :16, C_HI:C_HI + 128] = hi32
    dyn[:, :16, C_LO:C_LO + 32] = lo32
    return dyn  # [8, 128, NDYN]


def host_finish(reds):
    """reds: [8, 32, 40] -> out [128, 24]."""
    out = np.empty((NCORES * BPC, 2 * NQ), dtype=np.float32)
    for k in range(NCORES):
        red = reds[k]
        ex = np.empty((BPC, NQ), np.float32)
        ez = np.empty((BPC, NQ), np.float32)
        ex[:, 0:7] = red[0:7, 0:16].T
        ez[:, 0:7] = red[0:7, 16:32].T
        xlo = red[0:20, 32:36].reshape(4, 5, 4)   # [s4, w', m]
        zlo = red[0:20, 36:40].reshape(4, 5, 4)
        ex[:, 7:12] = xlo.transpose(2, 0, 1).reshape(16, 5)
        ez[:, 7:12] = zlo.transpose(2, 0, 1).reshape(16, 5)
        rows = slice(k * BPC, (k + 1) * BPC)
        out[rows, 0::2] = ex
        out[rows, 1::2] = ez
    return out


# ---------------- device program ----------------
_CACHE = {}


def _build_program():
    import concourse.bass as bass
    import concourse.mybir as mybir
    import concourse.tile as tile
    from concourse.tile_rust import add_dep_helper

    F32 = mybir.dt.float32
    BF16 = mybir.dt.bfloat16
    AXX = mybir.AxisListType.X
    ADD = mybir.AluOpType.add
    nc = bass.Bass()
    dyn_ext = nc.declare_dram_parameter("dyn", [128, NDYN], F32, isOutput=False)
    wc_ext = nc.declare_dram_parameter("wconst", [NWC, 128, 128], F32,
                                       isOutput=False)
    cp_ext = nc.declare_dram_parameter("cpack", [128, NCPK], F32,
                                       isOutput=False)
    red_ext = nc.declare_dram_parameter("red", [32, 40], F32, isOutput=True)

    with tile.TileContext(nc) as tc:
        with (
            tc.tile_pool(name="lpool", bufs=1) as lpool,
            tc.tile_pool(name="wpool", bufs=1) as wpool,
            tc.tile_pool(name="spool", bufs=2) as spool,
            tc.tile_pool(name="opool", bufs=1) as opool,
            tc.tile_pool(name="ppool", bufs=6, space="PSUM") as ppool,
            tc.tile_pool(name="tpool", bufs=2, space="PSUM") as tpool,
        ):
            last_dve = [None]       # newest DVE instr (chain target)
            last_pe = [None]        # newest non-ldweights PE instr
            pending_lds = []        # absorb lds awaiting a PE dependent
            dma_insts = []

            def dma(eng, **kw):
                dma_insts.append(eng.dma_start(**kw))
                return dma_insts[-1]

            def dve(fn, *a, **kw):
                # chained DVE op (must not read PSUM or landing DMAs):
                # dep on the previous DVE instr keeps the DVE queue in
                # creation order; the edge shares the DVE semaphore with
                # the op's data deps, so it costs nothing extra.
                i = fn(*a, **kw)
                if last_dve[0] is not None:
                    add_dep_helper(i.ins, last_dve[0].ins,
                                   reason="dve chain")
                last_dve[0] = i
                return i

            def dve_u(fn, *a, **kw):
                # PSUM-reading DVE op: its one wait is on the PE producer,
                # so no chain edge (would add a 2nd semaphore). A
                # ldweights immediately absorbs its tick for the PE queue.
                i = fn(*a, **kw)
                last_dve[0] = i
                ld = nc.tensor.ldweights(jw[:])
                add_dep_helper(ld.ins, i.ins, reason="absorb psum reader")
                pending_lds.append(ld)
                return i

            def copy(out, in_):
                return dve(nc.vector.tensor_copy, out, in_)

            def copy_ps(out, in_):
                return dve_u(nc.vector.tensor_copy, out, in_)

            # ---- land inputs; DVE-copy everything PE will read ----
            # Landing copies are unchained (their one wait is the DMA
            # queue sem); each gets an absorb ldweights so the PE queue
            # observes its exact DVE tick regardless of scheduling.
            jw = wpool.tile([128, 8], BF16, tag="jw")
            jwm = nc.vector.memset(jw[:], 0)
            last_dve[0] = jwm

            W = {}

            def land_in(ext_ap, shape, tagi):
                land = lpool.tile(shape, F32, tag=f"land{tagi}")
                dma(nc.sync, out=land[:], in_=ext_ap)
                t = wpool.tile(shape, F32, tag=f"t{tagi}")
                c = nc.vector.tensor_copy(t[:], land[:])
                last_dve[0] = c
                ld = nc.tensor.ldweights(jw[:])
                add_dep_helper(ld.ins, c.ins, reason="absorb landing copy")
                pending_lds.append(ld)
                return t

            for name, i in WIDX.items():
                W[name] = land_in(wc_ext[i], [128, 128], f"w{i}")
            cp = land_in(cp_ext[:], [128, NCPK], "cp")
            dyn = land_in(dyn_ext[:], [128, NDYN], "dyn")

            def absorb():
                ld = nc.tensor.ldweights(jw[:])
                if last_dve[0] is not None:
                    add_dep_helper(ld.ins, last_dve[0].ins,
                                   reason="absorb newest DVE tick")
                pending_lds.append(ld)

            def pe(fn, *a, **kw):
                i = fn(*a, **kw)
                for ld in pending_lds:
                    add_dep_helper(i.ins, ld.ins, reason="pe after absorbs")
                del pending_lds[:]
                if last_pe[0] is not None:
                    add_dep_helper(i.ins, last_pe[0].ins, reason="pe chain")
                last_pe[0] = i
                return i

            def cmm(ps, lhsT_list, rhs_list):
                n = len(lhsT_list)
                for k, (lt, rh) in enumerate(zip(lhsT_list, rhs_list)):
                    pe(nc.tensor.matmul, ps, lt, rh, start=(k == 0),
                       stop=(k == n - 1))

            # ---- G + st0 build ----
            G = wpool.tile([16, 512], F32, tag="G")
            lo_b = dyn[0:16, C_LO:C_LO + 32].unsqueeze(1).broadcast_to(
                (16, 16, 32))
            dve(nc.vector.tensor_mul,
                G[:].rearrange("r (s l) -> r s l", s=16, l=32),
                cp[0:16, 0:512].rearrange("r (s l) -> r s l", s=16, l=32),
                lo_b)
            hi_ap = dyn[0:16, C_HI:C_HI + 128]
            stA_r = spool.tile([128, 1024], F32, tag="stAr")
            absorb()
            for half in range(2):
                ps = ppool.tile([128, 512], F32, tag="ps")
                pe(nc.tensor.matmul, ps[:], hi_ap, G[:], start=True,
                   stop=True)
                copy_ps(stA_r[:, 512 * half:512 * half + 512], ps[:])
            stA_i = None

            # ---- A build: T = H^T per (L,c) ----
            A = {}
            col = C_A
            for L in range(NL):
                for c in range(2):
                    Tr, Ti = W['CHAINT'], None
                    for j in range(13):
                        lastj = (j == 12)
                        if lastj:
                            nTr = wpool.tile([128, 128], F32, tag=f"ArT{L}{c}")
                            nTi = wpool.tile([128, 128], F32, tag=f"AiT{L}{c}")
                        else:
                            nTr = spool.tile([128, 128], F32,
                                             tag=f"bT{j % 2}r")
                            nTi = spool.tile([128, 128], F32,
                                             tag=f"bT{j % 2}i")
                        t1 = spool.tile([128, 128], F32, tag="at1")
                        t2 = spool.tile([128, 128], F32, tag="at2")
                        if j % 2 == 0:
                            w = 6 - j // 2
                            dar = dyn[:, col + 0:col + 1]
                            dai = dyn[:, col + 1:col + 2]
                            dpr = dyn[:, col + 2:col + 3]
                            dpi = dyn[:, col + 3:col + 4]
                            col += 4
                            absorb()
                            Qr = tpool.tile([128, 128], F32, tag="pt")
                            pe(nc.tensor.matmul, Qr[:], W[f'X{w}'][:], Tr[:],
                               start=True, stop=True)
                            if Ti is None:
                                # Ti == 0: nTr = dar*Tr + dpr*Qr
                                dve(nc.vector.tensor_scalar_mul, t1[:], Tr[:],
                                    dar)
                                dve_u(nc.vector.tensor_scalar_mul, t2[:], Qr[:],
                                    dpr)
                                dve(nc.vector.tensor_add, nTr[:], t1[:], t2[:])
                                dve(nc.vector.tensor_scalar_mul, t1[:], Tr[:],
                                    dai)
                                dve_u(nc.vector.tensor_scalar_mul, t2[:], Qr[:],
                                    dpi)
                                dve(nc.vector.tensor_add, nTi[:], t1[:], t2[:])
                            else:
                                Qi = tpool.tile([128, 128], F32, tag="pt")
                                pe(nc.tensor.matmul, Qi[:], W[f'X{w}'][:],
                                   Ti[:], start=True, stop=True)
                                t3 = spool.tile([128, 128], F32, tag="at3")
                                t4 = spool.tile([128, 128], F32, tag="at4")
                                # nTr = dar*Tr - dai*Ti + dpr*Qr - dpi*Qi
                                dve(nc.vector.tensor_scalar_mul, t1[:], Tr[:],
                                    dar)
                                dve(nc.vector.tensor_scalar_mul, t2[:], Ti[:],
                                    dai)
                                dve(nc.vector.tensor_sub, t1[:], t1[:], t2[:])
                                dve_u(nc.vector.tensor_scalar_mul, t3[:], Qr[:],
                                    dpr)
                                dve_u(nc.vector.tensor_scalar_mul, t4[:], Qi[:],
                                    dpi)
                                dve(nc.vector.tensor_sub, t3[:], t3[:], t4[:])
                                dve(nc.vector.tensor_add, nTr[:], t1[:], t3[:])
                                # nTi = dar*Ti + dai*Tr + dpr*Qi + dpi*Qr
                                dve(nc.vector.tensor_scalar_mul, t1[:], Ti[:],
                                    dar)
                                dve(nc.vector.tensor_scalar_mul, t2[:], Tr[:],
                                    dai)
                                dve(nc.vector.tensor_add, t1[:], t1[:], t2[:])
                                dve_u(nc.vector.tensor_scalar_mul, t3[:], Qi[:],
                                    dpr)
                                dve_u(nc.vector.tensor_scalar_mul, t4[:], Qr[:],
                                    dpi)
                                dve(nc.vector.tensor_add, t3[:], t3[:], t4[:])
                                dve(nc.vector.tensor_add, nTi[:], t1[:], t3[:])
                        else:
                            w = 5 - j // 2  # CRX(w, w+1), perm X[w+1]
                            da = dyn[:, col + 0:col + 1]
                            si = dyn[:, col + 1:col + 2]
                            nsi = dyn[:, col + 2:col + 3]
                            col += 3
                            absorb()
                            Qr = tpool.tile([128, 128], F32, tag="pt")
                            Qi = tpool.tile([128, 128], F32, tag="pt")
                            pe(nc.tensor.matmul, Qr[:], W[f'X{w + 1}'][:],
                               Tr[:], start=True, stop=True)
                            pe(nc.tensor.matmul, Qi[:], W[f'X{w + 1}'][:],
                               Ti[:], start=True, stop=True)
                            # nTr = da*Tr + nsi*Qi ; nTi = da*Ti + si*Qr
                            dve(nc.vector.tensor_scalar_mul, t1[:], Tr[:], da)
                            dve_u(nc.vector.tensor_scalar_mul, t2[:], Qi[:], nsi)
                            dve(nc.vector.tensor_add, nTr[:], t1[:], t2[:])
                            dve(nc.vector.tensor_scalar_mul, t1[:], Ti[:], da)
                            dve_u(nc.vector.tensor_scalar_mul, t2[:], Qr[:], si)
                            dve(nc.vector.tensor_add, nTi[:], t1[:], t2[:])
                        Tr, Ti = nTr, nTi
                    Aneg = wpool.tile([128, 128], F32, tag=f"An{L}{c}")
                    dve(nc.vector.tensor_scalar_mul, Aneg[:], Ti[:], -1.0)
                    A[('rT', L, c)] = Tr
                    A[('iT', L, c)] = Ti
                    A[('negiT', L, c)] = Aneg

            # ---- R1 mats ----
            R1 = {}
            for L in range(NL):
                for c in range(2):
                    k = C_R1 + 3 * (2 * L + c)
                    tcos = wpool.tile([128, 128], F32, tag=f"r1c{L}{c}")
                    tsin = wpool.tile([128, 128], F32, tag=f"r1s{L}{c}")
                    tnsin = wpool.tile([128, 128], F32, tag=f"r1n{L}{c}")
                    dve(nc.vector.tensor_scalar_mul, tcos[:], W['ident'][:],
                        dyn[:, k:k + 1])
                    dve(nc.vector.tensor_scalar_mul, tsin[:], W['P56'][:],
                        dyn[:, k + 1:k + 2])
                    dve(nc.vector.tensor_scalar_mul, tnsin[:], W['P56'][:],
                        dyn[:, k + 2:k + 3])
                    R1[('cos', L, c)] = tcos
                    R1[('sinX', L, c)] = tsin
                    R1[('negsinX', L, c)] = tnsin

            # ---- M2 expand: I4 (x) M2 via IDG selector matmuls ----
            M2 = {}
            bi = 0
            absorb()
            for L in range(NL):
                for c in range(2):
                    for b6 in (0, 1):
                        for part in ('r', 'i'):
                            g, gp = bi // 6, bi % 6
                            absorb()
                            ps = tpool.tile([128, 128], F32, tag="pt")
                            for gq in range(4):
                                pe(nc.tensor.matmul,
                                   ps[:, 32 * gq:32 * gq + 32],
                                   cp[32 * g:32 * g + 32,
                                      CP_IDG + 128 * gq:CP_IDG + 128 * gq + 128],
                                   dyn[32 * g:32 * g + 32,
                                       C_M2 + 32 * gp:C_M2 + 32 * gp + 32],
                                   start=True, stop=True)
                            sm = wpool.tile([128, 128], F32,
                                            tag=f"sm{part}{L}{c}{b6}")
                            copy_ps(sm[:], ps[:])
                            M2[(part, L, c, b6)] = sm
                            bi += 1
                        smn = wpool.tile([128, 128], F32, tag=f"smn{L}{c}{b6}")
                        dve(nc.vector.tensor_scalar_mul, smn[:],
                            M2[('i', L, c, b6)][:], -1.0)
                        M2[('negi', L, c, b6)] = smn

            # ---- main loop ----
            for L in range(NL):
                stApost_r = spool.tile([128, 1024], F32, tag="sApr")
                stApost_i = spool.tile([128, 1024], F32, tag="sApi")
                for c in range(2):
                    absorb()
                    cols = slice(512 * c, 512 * (c + 1))
                    ps_r = ppool.tile([128, 512], F32, tag="ps")
                    ps_i = ppool.tile([128, 512], F32, tag="ps")
                    if L == 0:
                        cmm(ps_r[:], [A[('rT', L, c)][:]], [stA_r[:, cols]])
                        cmm(ps_i[:], [A[('iT', L, c)][:]], [stA_r[:, cols]])
                    else:
                        cmm(ps_r[:], [A[('rT', L, c)][:],
                                      A[('negiT', L, c)][:]],
                            [stA_r[:, cols], stA_i[:, cols]])
                        cmm(ps_i[:], [A[('iT', L, c)][:],
                                      A[('rT', L, c)][:]],
                            [stA_r[:, cols], stA_i[:, cols]])
                    copy_ps(stApost_r[:, cols], ps_r[:])
                    copy_ps(stApost_i[:, cols], ps_i[:])

                B0_r = spool.tile([128, 1024], F32, tag="B0r")
                B0_i = spool.tile([128, 1024], F32, tag="B0i")
                for m in range(8):
                    absorb()
                    cs = slice(128 * m, 128 * (m + 1))
                    for srct, dst in ((stApost_r, B0_r), (stApost_i, B0_i)):
                        pt = tpool.tile([128, 128], F32, tag="pt")
                        pe(nc.tensor.transpose, pt[:], srct[:, cs],
                           W['ident'][:])
                        copy_ps(dst[:, cs], pt[:])

                B0v_r = B0_r[:].rearrange("p (m h q) -> p m h q", m=8, h=32,
                                          q=4)
                B0v_i = B0_i[:].rearrange("p (m h q) -> p m h q", m=8, h=32,
                                          q=4)

                ps1 = {}
                for c in range(2):
                    absorb()
                    mc = slice(4 * c, 4 * (c + 1))
                    xr = B0v_r[:, mc, :, 1::2]
                    xi = B0v_i[:, mc, :, 1::2]
                    pr = ppool.tile([128, 4, 32, 2], F32, tag="ps")
                    pi = ppool.tile([128, 4, 32, 2], F32, tag="ps")
                    cmm(pr[:], [R1[('cos', L, c)][:], R1[('sinX', L, c)][:]],
                        [xr, xi])
                    cmm(pi[:], [R1[('cos', L, c)][:],
                                R1[('negsinX', L, c)][:]], [xi, xr])
                    ps1[c] = (pr, pi)

                B1_r = spool.tile([128, 1024], F32, tag="B1r")
                B1_i = spool.tile([128, 1024], F32, tag="B1i")
                B1v_r = B1_r[:].rearrange("p (m h q) -> p m h q", m=8, h=32,
                                          q=4)
                B1v_i = B1_i[:].rearrange("p (m h q) -> p m h q", m=8, h=32,
                                          q=4)
                for comp, B0v, B1v in ((0, B0v_r, B1v_r), (1, B0v_i, B1v_i)):
                    copy(B1v[:, :, :, 0], B0v[:, :, :, 0])
                    copy(B1v[:, :, :, 3], B0v[:, :, :, 2])
                    for c in range(2):
                        mc = slice(4 * c, 4 * (c + 1))
                        p = ps1[c][comp]
                        copy_ps(B1v[:, mc, :, 1], p[:, :, :, 0])
                        copy_ps(B1v[:, mc, :, 2], p[:, :, :, 1])

                B2_r = spool.tile([128, 1024], F32, tag="B2r")
                B2_i = spool.tile([128, 1024], F32, tag="B2i")
                B2v_r = B2_r[:].rearrange("p (m h q) -> p m h q", m=8, h=32,
                                          q=4)
                B2v_i = B2_i[:].rearrange("p (m h q) -> p m h q", m=8, h=32,
                                          q=4)
                for c in range(2):
                    mc = slice(4 * c, 4 * (c + 1))
                    for b6 in (0, 1):
                        absorb()
                        qs = slice(b6, 4, 2)
                        xr = B1v_r[:, mc, :, qs]
                        xi = B1v_i[:, mc, :, qs]
                        pr = ppool.tile([128, 4, 32, 2], F32, tag="ps")
                        pi = ppool.tile([128, 4, 32, 2], F32, tag="ps")
                        cmm(pr[:], [M2[('r', L, c, b6)][:],
                                    M2[('negi', L, c, b6)][:]], [xr, xi])
                        cmm(pi[:], [M2[('i', L, c, b6)][:],
                                    M2[('r', L, c, b6)][:]], [xr, xi])
                        copy_ps(B2v_r[:, mc, :, qs], pr[:])
                        copy_ps(B2v_i[:, mc, :, qs], pi[:])

                if L < NL - 1:
                    stA_r = spool.tile([128, 1024], F32, tag="stAr")
                    stA_i = spool.tile([128, 1024], F32, tag="stAi")
                    for m in range(8):
                        absorb()
                        cs = slice(128 * m, 128 * (m + 1))
                        for src, dst in ((B2_r, stA_r), (B2_i, stA_i)):
                            pt = tpool.tile([128, 128], F32, tag="pt")
                            pe(nc.tensor.transpose, pt[:], src[:, cs],
                               W['ident'][:])
                            copy_ps(dst[:, cs], pt[:])

            # ---- endgame ----
            red = opool.tile([32, 40], F32, tag="red")
            dve(nc.vector.memset, red[:], 0)

            # circuit 1 (Z), layout B
            sq_t1 = spool.tile([128, 512], F32, tag="sqt1")
            sq_t2 = spool.tile([128, 512], F32, tag="sqt2")
            sq_z = spool.tile([128, 512], F32, tag="sqz")
            dve(nc.vector.tensor_mul, sq_t1[:], B2_r[:, 512:], B2_r[:, 512:])
            dve(nc.vector.tensor_mul, sq_t2[:], B2_i[:, 512:], B2_i[:, 512:])
            dve(nc.vector.tensor_add, sq_z[:], sq_t1[:], sq_t2[:])
            absorb()
            psl = ppool.tile([20, 512], F32, tag="ps")
            pe(nc.tensor.matmul, psl[:], cp[:, CP_SL20:CP_SL20 + 20],
               sq_z[:], start=True, stop=True)
            dve_u(nc.vector.tensor_reduce, red[0:20, 36:40],
                psl[:].rearrange("p (g h) -> p g h", g=4, h=128), AXX, ADD)
            sqzA = spool.tile([128, 512], F32, tag="sqzA")
            for m in range(4):
                absorb()
                cs = slice(128 * m, 128 * (m + 1))
                pt = tpool.tile([128, 128], F32, tag="pt")
                pe(nc.tensor.transpose, pt[:], sq_z[:, cs], W['ident'][:])
                copy_ps(sqzA[:, cs], pt[:])
            absorb()
            psh = ppool.tile([8, 512], F32, tag="ps")
            pe(nc.tensor.matmul, psh[:], cp[:, CP_SHX:CP_SHX + 8], sqzA[:],
               start=True, stop=True)
            dve_u(nc.vector.tensor_reduce, red[0:8, 16:32],
                psh[:].rearrange("p (n l) -> p n l", n=16, l=32), AXX, ADD)

            # circuit 0 (X): back to layout A, Hhi, squares
            fA_r = spool.tile([128, 512], F32, tag="fAr")
            fA_i = spool.tile([128, 512], F32, tag="fAi")
            for m in range(4):
                absorb()
                cs = slice(128 * m, 128 * (m + 1))
                for src, dst in ((B2_r, fA_r), (B2_i, fA_i)):
                    pt = tpool.tile([128, 128], F32, tag="pt")
                    pe(nc.tensor.transpose, pt[:], src[:, cs], W['ident'][:])
                    copy_ps(dst[:, cs], pt[:])
            absorb()
            ph_r = ppool.tile([128, 512], F32, tag="ps")
            ph_i = ppool.tile([128, 512], F32, tag="ps")
            cmm(ph_r[:], [W['Hhi'][:]], [fA_r[:]])
            cmm(ph_i[:], [W['Hhi'][:]], [fA_i[:]])
            phs_r = spool.tile([128, 512], F32, tag="phsr")
            phs_i = spool.tile([128, 512], F32, tag="phsi")
            copy_ps(phs_r[:], ph_r[:])
            copy_ps(phs_i[:], ph_i[:])
            sq_x = spool.tile([128, 512], F32, tag="sqx")
            dve(nc.vector.tensor_mul, sq_t1[:], phs_r[:], phs_r[:])
            dve(nc.vector.tensor_mul, sq_t2[:], phs_i[:], phs_i[:])
            dve(nc.vector.tensor_add, sq_x[:], sq_t1[:], sq_t2[:])
            absorb()
            psh2 = ppool.tile([8, 512], F32, tag="ps")
            pe(nc.tensor.matmul, psh2[:], cp[:, CP_SHX:CP_SHX + 8], sq_x[:],
               start=True, stop=True)
            dve_u(nc.vector.tensor_reduce, red[0:8, 0:16],
                psh2[:].rearrange("p (n l) -> p n l", n=16, l=32), AXX, ADD)
            sqxB = spool.tile([128, 512], F32, tag="sqxB")
            for m in range(4):
                absorb()
                cs = slice(128 * m, 128 * (m + 1))
                pt = tpool.tile([128, 128], F32, tag="pt")
                pe(nc.tensor.transpose, pt[:], sq_x[:, cs], W['ident'][:])
                copy_ps(sqxB[:, cs], pt[:])
            absorb()
            psl2 = ppool.tile([20, 512], F32, tag="ps")
            pe(nc.tensor.matmul, psl2[:], cp[:, CP_SL20:CP_SL20 + 20],
               sqxB[:], start=True, stop=True)
            last_red = dve_u(nc.vector.tensor_reduce, red[0:20, 32:36],
                           psl2[:].rearrange("p (g h) -> p g h", g=4, h=128),
                           AXX, ADD)
            dma(nc.gpsimd, out=red_ext[:], in_=red[:])
            final_pe = pe(nc.tensor.ldweights, jw[:])

            finale = [last_red, final_pe] + dma_insts[-14:]
            for depi in finale:
                n = nc.sync.nop()
                add_dep_helper(n.ins, depi.ins, reason="tail tick absorb")

    return nc


def _get_program():
    if 'prog' not in _CACHE:
        _CACHE['prog'] = _build_program()
    return _CACHE['prog']


# ---------------- host <-> device glue ----------------
def _get_runner(nc):
    if 'runner' in _CACHE:
        return _CACHE['runner']
    import jax
    from jax.sharding import Mesh, PartitionSpec, NamedSharding
    from jax.experimental.shard_map import shard_map
    from concourse import bass2jax, mybir
    bass2jax.install_neuronx_cc_hook()
    _p = bass2jax._bass_exec_p

    pname = nc.partition_id_tensor.name if nc.partition_id_tensor else None
    in_names, out_names, out_avals, zero_outs = [], [], [], []
    for alloc in nc.m.functions[0].allocations:
        if not isinstance(alloc, mybir.MemoryLocationSet):
            continue
        name = alloc.memorylocations[0].name
        if alloc.kind == "ExternalInput":
            if name != pname:
                in_names.append(name)
        elif alloc.kind == "ExternalOutput":
            shape = tuple(alloc.tensor_shape)
            dtype = mybir.dt.np(alloc.dtype)
            out_names.append(name)
            out_avals.append(jax.core.ShapedArray(shape, dtype))
            zero_outs.append(np.zeros(shape, dtype))
    n_params = len(in_names)
    n_outs = len(out_avals)
    all_names = in_names + out_names
    if pname is not None:
        all_names = all_names + [pname]
    donate = tuple(range(n_params, n_params + n_outs))

    def _body(*args):
        operands = list(args)
        if pname is not None:
            operands.append(bass2jax.partition_id_tensor())
        outs = _p.bind(
            *operands, out_avals=tuple(out_avals), in_names=tuple(all_names),
            out_names=tuple(out_names), lowering_input_output_aliases=(),
            sim_require_finite=True, sim_require_nnan=True, nc=nc)
        return tuple(outs)

    devices = jax.devices()[:NCORES]
    mesh = Mesh(np.asarray(devices), ("core",))
    in_specs = (PartitionSpec("core"),) * (n_params + n_outs)
    out_specs = (PartitionSpec("core"),) * n_outs
    sharded = jax.jit(
        shard_map(_body, mesh=mesh, in_specs=in_specs, out_specs=out_specs,
                  check_rep=False),
        donate_argnums=donate, keep_unused=True)

    # commit input-independent constants to the devices once
    sh = NamedSharding(mesh, PartitionSpec("core"))
    wconst, cpack = build_constants()
    committed = {
        'wconst': jax.device_put(
            np.concatenate([wconst] * NCORES, axis=0), sh),
        'cpack': jax.device_put(np.concatenate([cpack] * NCORES, axis=0), sh),
    }
    zo_full = [np.concatenate([z] * NCORES, axis=0) for z in zero_outs]

    def run(dyn):
        ins = []
        for n in in_names:
            if n == 'dyn':
                ins.append(dyn.reshape(NCORES * 128, NDYN))
            else:
                ins.append(committed[n])
        outs = sharded(*ins, *zo_full)
        arr = np.asarray(outs[out_names.index('red')])
        return arr.reshape(NCORES, 32, 40)

    _CACHE['runner'] = run
    return run


def kernel(x, rotations, cx_strengths, t_gates, _run_kwargs=None):
    dyn = host_prep(x, rotations, cx_strengths, t_gates)
    nc = _get_program()
    reds = _get_runner(nc)(dyn)
    return host_finish(reds)

